# revision 1
# baseline (speedup 1.0000x reference)
"""Trainium2 Bass kernel for single-head causal attention (B=4, T=4096, C=2048, HS=128).

Sharding: 2 cores per batch element (8 cores, B=4). Each core owns 2048
sequence rows of its batch element, arranged (by the host) as interleaved
512-row q-tiles:
  role A (even cores): global q-tiles [0, 2, 4, 6]  (slot s <-> tile 2s)
  role B (odd cores):  global q-tiles [1, 3, 5, 7]  (slot s <-> tile 2s+1)
Each core projects Q^T/K^T/V^T for its own rows (contraction over C with W
stationary), AllGathers K^T/V^T within its pair, DMA-transposes
V^T -> V, then runs causal attention over its 4 q-slots with static
per-slot k-extents [8, 16, 24, 32] k-tiles of 128. All role differences are
input data (row order + mask tiles), so all 8 cores run one SPMD graph.
Softmax skips the row-max (scores are O(1) by construction); causal masking
multiplies exp values by {0,1} mask tiles in the last 8 k-tiles of a slot.

Global 512-row k-chunk g lives at role g%2, block g//2 of the gathered
buffers (kb -> role (kb//4)%2, half (kb//8)//2, tile (kb//8)%2*4 + kb%4).
"""

import math

import numpy as np
import ml_dtypes

import concourse.bacc as bacc
import concourse.tile as tile
from concourse import mybir
from concourse.bass_utils import run_bass_kernel_spmd

B, T, C, HS = 4, 4096, 2048, 128
NCORES = 8
TOWN = 2048              # sequence rows owned per core
NCT = C // 128           # 16 contraction tiles
QTILES_A = [0, 2, 4, 6]
QTILES_B = [1, 3, 5, 7]
ES = [8, 16, 24, 32]     # static per-slot k extents (k-tiles of 128)

BF16 = ml_dtypes.bfloat16


def build_graph(with_collective=True):
    nc = bacc.Bacc(
        "TRN2", target_bir_lowering=False, debug=False, num_devices=NCORES
    )
    bf = mybir.dt.bfloat16
    f32 = mybir.dt.float32

    xt_d = nc.dram_tensor("xt", [128, NCT, TOWN], bf, kind="ExternalInput")
    w3_d = nc.dram_tensor("w3", [128, 3, NCT, HS], bf, kind="ExternalInput")
    msk_d = nc.dram_tensor("msk", [128, 8, 512], bf, kind="ExternalInput")
    idn_d = nc.dram_tensor("idn", [128, 128], f32, kind="ExternalInput")
    onc_d = nc.dram_tensor("onc", [128, 1], bf, kind="ExternalInput")
    onr_d = nc.dram_tensor("onr", [1, 128], bf, kind="ExternalInput")
    # stored as [slot, q_lo, u, hs]; host un-permutes (q = u*128 + q_lo)
    out_d = nc.dram_tensor("out", [4, 128, 4, HS], f32, kind="ExternalOutput")

    with tile.TileContext(nc) as tc:
        with (
            tc.tile_pool(name="big", bufs=1) as big,
            tc.tile_pool(name="dram", bufs=1, space="DRAM") as dram,
        ):
            # ---- loads, spread across the three DMA paths ----
            w3 = big.tile([128, 3, NCT, HS], bf, tag="w3")
            xt = big.tile([128, NCT, TOWN], bf, tag="xt")
            # SP queue: x chunks 0,3,6 ; ACT queue: wk/wv then 1,4,7 ;
            # Pool queue: 2,5 then wq and the small aux tensors.
            def xchunk(eng, ch):
                eng.dma_start(
                    xt[:, ch * 2 : (ch + 1) * 2, :],
                    xt_d[:, ch * 2 : (ch + 1) * 2, :],
                )

            nc.scalar.dma_start(w3[:, 1:3, 0:4, :], w3_d[:, 1:3, 0:4, :])
            xchunk(nc.sync, 0)
            xchunk(nc.scalar, 1)
            xchunk(nc.gpsimd, 2)
            nc.sync.dma_start(w3[:, 1:3, 4:16, :], w3_d[:, 1:3, 4:16, :])
            xchunk(nc.gpsimd, 3)
            xchunk(nc.scalar, 4)
            xchunk(nc.sync, 5)
            xchunk(nc.scalar, 6)
            xchunk(nc.sync, 7)
            # aux tensors load after the x chunks so they don't steal
            # DMA bandwidth from the projection c-loop
            nc.scalar.dma_start(w3[:, 0:1, :, :], w3_d[:, 0:1, :, :])
            msk = big.tile([128, 8, 512], bf, tag="msk")
            nc.scalar.dma_start(msk[:], msk_d[:])
            idn = big.tile([128, 128], f32, tag="idn")
            nc.gpsimd.dma_start(idn[:], idn_d[:])
            onc = big.tile([128, 1], bf, tag="onc")
            nc.gpsimd.dma_start(onc[:], onc_d[:])
            onr = big.tile([1, 128], bf, tag="onr")
            nc.gpsimd.dma_start(onr[:], onr_d[:])

            ktq = big.tile([128, TOWN], bf, tag="ktq")  # own K^T (slot order)
            vtq = big.tile([128, TOWN], bf, tag="vtq")  # own V^T
            qts = [
                big.tile([128, 512], bf, tag=f"qt{s}", name=f"qt{s}")
                for s in range(4)
            ]
            # gathered K^T / V per (role, half)
            ktf = [
                [
                    big.tile([128, 1024], bf, tag=f"ktf{r}{h}", name=f"ktf{r}{h}")
                    for h in range(2)
                ]
                for r in range(2)
            ]
            v3 = [
                [
                    big.tile([128, 8, HS], bf, tag=f"v3{r}{h}", name=f"v3{r}{h}")
                    for h in range(2)
                ]
                for r in range(2)
            ]

            kvb = dram.tile([256, TOWN], bf, tag="kvb")
            kvg = dram.tile([512, TOWN], bf, tag="kvg")

            def allgather():
                if with_collective:
                    nc.gpsimd.collective_compute(
                        "AllGather",
                        mybir.AluOpType.bypass,
                        replica_groups=[[0, 1], [2, 3], [4, 5], [6, 7]],
                        ins=[kvb.opt()],
                        outs=[kvg.opt()],
                    )
                else:  # timeline-model stub: same data volume, no comms
                    nc.sync.dma_start(kvg[0:256, :], kvb[:])
                    nc.sync.dma_start(kvg[256:512, :], kvb[:])

            def unpack(h):
                for r in range(2):
                    nc.sync.dma_start(
                        ktf[r][h][:],
                        kvg[r * 256 : r * 256 + 128, h * 1024 : (h + 1) * 1024],
                    )
                    nc.sync.dma_start_transpose(
                        v3[r][h][:],
                        kvg[
                            r * 256 + 128 : r * 256 + 256,
                            h * 1024 : (h + 1) * 1024,
                        ],
                    )

            # ---- projections ----
            with tc.tile_pool(name="pjps", bufs=8, space="PSUM") as pjps:
                # PE warm-up: dependency-free matmuls on a zeroed tile keep
                # the HAM clock-gate warm while the first input DMAs land.
                wut = big.tile([128, 512], bf, tag="wut")
                nc.vector.memset(wut[:], 0.0)
                wup = pjps.tile([128, 512], f32, tag="pj", name="wup")
                for _ in range(28):
                    nc.tensor.matmul(
                        wup[:], wut[:, 0:128], wut[:], start=True, stop=True
                    )
                # K,V for all t-chunks in one c-pass (keeps PE ahead of
                # the serial HBM delivery), then both AllGathers.
                ps8 = [
                    pjps.tile([128, 512], f32, tag="pj", name=f"pa{i}")
                    for i in range(8)
                ]
                for c in range(NCT):
                    for wi in range(2):  # 0 = K, 1 = V
                        for t4 in range(4):
                            nc.tensor.matmul(
                                ps8[wi * 4 + t4][:],
                                w3[:, wi + 1, c, :],
                                xt[:, c, t4 * 512 : (t4 + 1) * 512],
                                start=(c == 0),
                                stop=(c == NCT - 1),
                            )
                for wi, dest in [(0, ktq), (1, vtq)]:
                    for t4 in range(4):
                        nc.vector.tensor_copy(
                            dest[:, t4 * 512 : (t4 + 1) * 512],
                            ps8[wi * 4 + t4][:],
                        )
                nc.sync.dma_start(kvb[0:128, :], ktq[:])
                nc.sync.dma_start(kvb[128:256, :], vtq[:])
                allgather()

                def qt_phase(slots):
                    pq = [
                        pjps.tile([128, 512], f32, tag="pj", name=f"pq{t4}")
                        for t4 in slots
                    ]
                    for c in range(NCT):
                        for i, t4 in enumerate(slots):
                            nc.tensor.matmul(
                                pq[i][:],
                                w3[:, 0, c, :],
                                xt[:, c, t4 * 512 : (t4 + 1) * 512],
                                start=(c == 0),
                                stop=(c == NCT - 1),
                            )
                    for i, t4 in enumerate(slots):
                        nc.vector.tensor_copy(qts[t4][:], pq[i][:])

                qt_phase([0, 1])
                unpack(0)
                qt_phase([2, 3])
                unpack(1)

            # ---- attention ----
            with (
                tc.tile_pool(name="ops", bufs=1, space="PSUM") as ops,
                tc.tile_pool(name="dps", bufs=1, space="PSUM") as dps,
                tc.tile_pool(name="sps", bufs=4, space="PSUM") as sps,
                tc.tile_pool(name="tps", bufs=2, space="PSUM") as tps,
                tc.tile_pool(name="pp", bufs=5) as pp,
                tc.tile_pool(name="ep", bufs=2) as ep,
                tc.tile_pool(name="yp", bufs=2) as yp,
            ):
                for s in range(4):
                    E = ES[s]
                    o_ps = ops.tile([128, 512], f32, tag="o")
                    d_ps = dps.tile([1, 512], f32, tag="d")
                    for kb in range(E):
                        r = (kb // 4) % 2
                        blk = kb // 8
                        half, ti = blk // 2, (blk % 2) * 4 + kb % 4
                        j = kb - (E - 8)
                        s_ps = sps.tile([128, 512], f32, tag="s")
                        nc.tensor.matmul(
                            s_ps[:],
                            ktf[r][half][:, ti * 128 : (ti + 1) * 128],
                            qts[s][:],
                            start=True,
                            stop=True,
                        )
                        p_sb = pp.tile([128, 512], bf, tag="p")
                        nc.scalar.activation(
                            p_sb[:], s_ps[:], mybir.ActivationFunctionType.Exp
                        )
                        if j >= 0:
                            nc.vector.tensor_mul(p_sb[:], p_sb[:], msk[:, j, :])
                        nc.tensor.matmul(
                            o_ps[:],
                            v3[r][half][:, ti, :],
                            p_sb[:],
                            start=(kb == 0),
                            stop=(kb == E - 1),
                            skip_group_check=True,
                        )
                        nc.tensor.matmul(
                            d_ps[:],
                            onc[:],
                            p_sb[:],
                            start=(kb == 0),
                            stop=(kb == E - 1),
                            skip_group_check=True,
                        )
                    # epilogue: normalize, DMA-transpose (bf16), store with cast
                    ot = ep.tile([128, 512], f32, tag="ot")
                    nc.vector.tensor_copy(ot[:], o_ps[:])
                    den = ep.tile([1, 512], f32, tag="den")
                    nc.vector.tensor_copy(den[:], d_ps[:])
                    rec = ep.tile([1, 512], f32, tag="rec")
                    nc.vector.reciprocal(rec[:], den[:])
                    rb = ep.tile([128, 512], f32, tag="rb")
                    nc.gpsimd.partition_broadcast(rb[:], rec[:])
                    nc.vector.tensor_mul(ot[:], ot[:], rb[:])
                    y4 = yp.tile([128, 4, 128], f32, tag="y")
                    for u in range(4):
                        tp = tps.tile([128, 128], f32, tag="tp")
                        nc.tensor.transpose(
                            tp[:], ot[:, u * 128 : (u + 1) * 128], idn[:]
                        )
                        nc.vector.tensor_copy(y4[:, u, :], tp[:])
                    nc.gpsimd.dma_start(out_d[s], y4[:])

    nc.compile()
    return nc


def _role_qtiles(h):
    return QTILES_A if h == 0 else QTILES_B


def _make_masks(h):
    """[128, 8, 512] bf16 mask tiles for the last 8 k-tiles of every slot."""
    m = np.zeros((128, 8, 512), np.float32)
    k = np.arange(128)[:, None]
    q = np.arange(512)[None, :]
    for j in range(8):
        if h == 0:  # role A: diag at j<4, zero at j>=4
            if j < 4:
                m[:, j, :] = (128 * j + k <= q).astype(np.float32)
        else:  # role B: ones at j<4, diag at j>=4
            if j < 4:
                m[:, j, :] = 1.0
            else:
                m[:, j, :] = (128 * (j - 4) + k <= q).astype(np.float32)
    return m.astype(BF16)


def make_in_maps(x, Wq, Wk, Wv):
    """Host-side sharding + layout prep. x [B,T,C] f32, W* [C,HS] f32."""
    wq_s = np.asarray(Wq, np.float32) / math.sqrt(HS)
    # [3, C, HS] -> [128, 3, NCT, HS] with row c = ci*128 + p
    w3 = np.stack(
        [wq_s, np.asarray(Wk, np.float32), np.asarray(Wv, np.float32)]
    )
    w3_arr = np.ascontiguousarray(
        w3.reshape(3, NCT, 128, HS).transpose(2, 0, 1, 3)
    ).astype(BF16)
    idn = np.eye(128, dtype=np.float32)
    onc = np.ones((128, 1), np.float32).astype(BF16)
    onr = np.ones((1, 128), np.float32).astype(BF16)
    msk_by_role = [_make_masks(0), _make_masks(1)]

    in_maps = []
    for core in range(NCORES):
        b, h = core // 2, core % 2
        qtiles = _role_qtiles(h)
        rows = np.concatenate(
            [np.arange(g * 512, (g + 1) * 512) for g in qtiles]
        )
        xr = np.asarray(x[b])[rows]  # [2048 rows, C] f32
        xT = np.ascontiguousarray(xr.T).astype(BF16)  # [C, 2048]
        xt_arr = np.ascontiguousarray(
            xT.reshape(NCT, 128, TOWN).transpose(1, 0, 2)
        )  # [128, NCT, 2048]
        in_maps.append(
            {
                "xt": xt_arr,
                "w3": w3_arr,
                "msk": msk_by_role[h],
                "idn": idn,
                "onc": onc,
                "onr": onr,
            }
        )
    return in_maps


def assemble_out(results):
    """results: list of 8 dicts with 'out' [4,128,4,HS] -> y [B,T,HS] f32."""
    y = np.zeros((B, T, HS), np.float32)
    for core in range(NCORES):
        b, h = core // 2, core % 2
        qtiles = _role_qtiles(h)
        o = np.asarray(results[core]["out"])  # [4, 128, 4, HS]
        o = o.transpose(0, 2, 1, 3).reshape(4, 512, HS)  # q = u*128 + q_lo
        for s in range(4):
            g = qtiles[s]
            y[b, g * 512 : (g + 1) * 512] = o[s]
    return y


_NC_CACHE = None


def _get_graph():
    global _NC_CACHE
    if _NC_CACHE is None:
        _NC_CACHE = build_graph()
    return _NC_CACHE


def kernel(x, Wq, Wk, Wv):
    import time

    nc = _get_graph()
    in_maps = make_in_maps(x, Wq, Wk, Wv)
    try:
        res = run_bass_kernel_spmd(nc, in_maps, list(range(NCORES)))
    except Exception:
        time.sleep(15)  # transient device/mesh hiccup: one retry
        res = run_bass_kernel_spmd(nc, in_maps, list(range(NCORES)))
    return assemble_out(res.results)



# revision 6
# speedup vs baseline: 48.1460x; 48.1460x over previous
"""Trainium2 Bass kernel for single-head causal attention (B=4, T=4096, C=2048, HS=128).

Sharding: 2 cores per batch element (8 cores, B=4), interleaved 512-row
q-chunks: role A (even cores) owns global chunks [0,2,4,6], role B (odd)
[1,3,5,7]. Each core projects Q^T/K^T/V^T for its own 2048 rows, AllGathers
K^T/V^T within its pair via a DRAM staging buffer, and runs causal
attention over its 4 q-slots with static per-slot extents of 2(s+1)
512-chunks.

Attention is split own-chunks-first / partner-chunks-second so the
AllGather overlaps Q projections + the own half of attention. Scores for
3 k-tiles at a time land in a 3-bank PSUM tile and are exponentiated by a
single batched ACT instruction; AV partials and column-packed (tile_position)
denominator partials accumulate per group in a 2-bank PSUM tile and are
folded into per-slot SBUF accumulators by the vector engine. Softmax
normalization uses a broadcast matmul (ones/32 stationary) +
reciprocal_approx_fast. The output is written as y^T [slot, hs, 512] and
transposed on the host. Partner data addressing is role-independent: both
parities of the gathered buffer are loaded and blended with per-core 0/1
selectors, so all 8 cores run one SPMD graph.
"""

import math

import numpy as np
import ml_dtypes

import concourse.bacc as bacc
import concourse.tile as tile
from concourse import mybir
from concourse.bass_utils import run_bass_kernel_spmd

B, T, C, HS = 4, 4096, 2048, 128
NCORES = 8
TOWN = 2048              # sequence rows owned per core
NCT = C // 128           # 16 contraction tiles
QTILES_A = [0, 2, 4, 6]
QTILES_B = [1, 3, 5, 7]

BF16 = ml_dtypes.bfloat16


def build_graph(with_collective=True):
    nc = bacc.Bacc(
        "TRN2", target_bir_lowering=False, debug=False, num_devices=NCORES
    )
    bf = mybir.dt.bfloat16
    f32 = mybir.dt.float32
    EXP = mybir.ActivationFunctionType.Exp

    xt_d = nc.dram_tensor("xt", [128, NCT, TOWN], bf, kind="ExternalInput")
    w3_d = nc.dram_tensor("w3", [128, 3, NCT, HS], bf, kind="ExternalInput")
    mo_d = nc.dram_tensor("mo", [128, 2048], bf, kind="ExternalInput")
    mp_d = nc.dram_tensor("mp", [128, 2048], bf, kind="ExternalInput")
    ps0_d = nc.dram_tensor("ps0", [128, 1], f32, kind="ExternalInput")
    ps1_d = nc.dram_tensor("ps1", [128, 1], f32, kind="ExternalInput")
    # y^T per slot, normalized; host transposes to [512, HS]
    out_d = nc.dram_tensor("out", [4, 128, 512], f32, kind="ExternalOutput")

    with tile.TileContext(nc) as tc:
        with (
            tc.tile_pool(name="big", bufs=1) as big,
            tc.tile_pool(name="dram", bufs=1, space="DRAM") as dram,
        ):
            # ---- loads, spread across the three DMA paths ----
            w3 = big.tile([128, 3, NCT, HS], bf, tag="w3")
            xt = big.tile([128, NCT, TOWN], bf, tag="xt")

            def xchunk(eng, ch):
                eng.dma_start(
                    xt[:, ch * 2 : (ch + 1) * 2, :],
                    xt_d[:, ch * 2 : (ch + 1) * 2, :],
                )

            nc.scalar.dma_start(w3[:, 1:3, 0:4, :], w3_d[:, 1:3, 0:4, :])
            xchunk(nc.sync, 0)
            xchunk(nc.scalar, 1)
            xchunk(nc.gpsimd, 2)
            nc.sync.dma_start(w3[:, 1:3, 4:16, :], w3_d[:, 1:3, 4:16, :])
            xchunk(nc.gpsimd, 3)
            xchunk(nc.scalar, 4)
            xchunk(nc.sync, 5)
            xchunk(nc.scalar, 6)
            xchunk(nc.sync, 7)
            nc.scalar.dma_start(w3[:, 0:1, :, :], w3_d[:, 0:1, :, :])
            mo = big.tile([128, 2048], bf, tag="mo")
            nc.scalar.dma_start(mo[:], mo_d[:])
            mp = big.tile([128, 2048], bf, tag="mp")
            nc.scalar.dma_start(mp[:], mp_d[:])
            ps0 = big.tile([128, 1], f32, tag="ps0")
            nc.gpsimd.dma_start(ps0[:], ps0_d[:])
            ps1 = big.tile([128, 1], f32, tag="ps1")
            nc.gpsimd.dma_start(ps1[:], ps1_d[:])

            # ---- constants ----
            wut = big.tile([128, 512], bf, tag="wut")
            nc.vector.memset(wut[:], 0.0)
            on32 = big.tile([128, 32], bf, tag="on32")
            nc.vector.memset(on32[:], 1.0)
            scb = big.tile([128, 128], f32, tag="scb")
            nc.vector.memset(scb[:], 1.0 / 32.0)

            # ---- persistent SBUF tensors ----
            ktq = big.tile([128, TOWN], bf, tag="ktq")  # own K^T (slot order)
            vtq = big.tile([128, TOWN], bf, tag="vtq")  # own V^T
            v3o = big.tile([128, 16, HS], bf, tag="v3o")  # own V (k-major)
            qts = [
                big.tile([128, 512], bf, tag=f"qt{s}", name=f"qt{s}")
                for s in range(4)
            ]
            ktp = [
                big.tile([128, TOWN], bf, tag=f"ktp{r}", name=f"ktp{r}")
                for r in range(2)
            ]
            v3p = [
                big.tile([128, 16, HS], bf, tag=f"v3p{r}", name=f"v3p{r}")
                for r in range(2)
            ]
            ktpar = big.tile([128, TOWN], bf, tag="ktpar")
            v3par = big.tile([128, 16, HS], bf, tag="v3par")
            accs = [
                big.tile([128, 1024], f32, tag=f"acc{s}", name=f"acc{s}")
                for s in range(4)
            ]

            kvb = dram.tile([256, TOWN], bf, tag="kvb")
            kvg = dram.tile([512, TOWN], bf, tag="kvg")

            # ---- projections: K,V for own rows ----
            with tc.tile_pool(name="pjps", bufs=8, space="PSUM") as pjps:
                # PE warm-up while the first input DMAs land
                wup = pjps.tile([128, 512], f32, tag="pj", name="wup")
                for _ in range(16):
                    nc.tensor.matmul(
                        wup[:], wut[:, 0:128], wut[:], start=True, stop=True
                    )
                ps8 = [
                    pjps.tile([128, 512], f32, tag="pj", name=f"pa{i}")
                    for i in range(8)
                ]
                for c in range(NCT):
                    for wi in range(2):  # 0 = K, 1 = V
                        for t4 in range(4):
                            nc.tensor.matmul(
                                ps8[wi * 4 + t4][:],
                                w3[:, wi + 1, c, :],
                                xt[:, c, t4 * 512 : (t4 + 1) * 512],
                                start=(c == 0),
                                stop=(c == NCT - 1),
                            )
                for wi, dest in [(0, ktq), (1, vtq)]:
                    for t4 in range(4):
                        nc.vector.tensor_copy(
                            dest[:, t4 * 512 : (t4 + 1) * 512],
                            ps8[wi * 4 + t4][:],
                        )
                nc.sync.dma_start(kvb[0:128, :], ktq[:])
                nc.sync.dma_start(kvb[128:256, :], vtq[:])
                if with_collective:
                    nc.gpsimd.collective_compute(
                        "AllGather",
                        mybir.AluOpType.bypass,
                        replica_groups=[[0, 1], [2, 3], [4, 5], [6, 7]],
                        ins=[kvb.opt()],
                        outs=[kvg.opt()],
                    )
                else:  # timeline-model stub: same data volume, no comms
                    nc.scalar.dma_start(kvg[0:256, :], kvb[:])
                    nc.scalar.dma_start(kvg[256:512, :], kvb[:])
                # own V: transpose V^T from the DRAM staging buffer
                nc.sync.dma_start_transpose(v3o[:], kvb[128:256, :])
                # partner halves (block on the collective; dedicated queues)
                nc.gpsimd.dma_start(ktp[0][:], kvg[0:128, :])
                nc.gpsimd.dma_start(ktp[1][:], kvg[256:384, :])
                nc.sync.dma_start_transpose(v3p[0][:], kvg[128:256, :])
                nc.sync.dma_start_transpose(v3p[1][:], kvg[384:512, :])

            # role-independent partner buffers: blend the two parities with
            # per-core 0/1 selectors (ps0 = partner-is-parity-0). Emitted
            # between the own and partner phases so the vector queue is not
            # head-of-line blocked on the collective during the own phase.
            def emit_blends():
                nc.vector.tensor_scalar_mul(ktp[0][:], ktp[0][:], ps0[:])
                nc.vector.tensor_scalar_mul(ktp[1][:], ktp[1][:], ps1[:])
                nc.vector.tensor_add(ktpar[:], ktp[0][:], ktp[1][:])
                nc.vector.tensor_scalar_mul(v3p[0][:], v3p[0][:], ps0[:])
                nc.vector.tensor_scalar_mul(v3p[1][:], v3p[1][:], ps1[:])
                nc.vector.tensor_add(v3par[:], v3p[0][:], v3p[1][:])

            # ---- attention ----
            with (
                tc.tile_pool(name="srng", bufs=2, space="PSUM") as srng,
                tc.tile_pool(name="pps", bufs=1, space="PSUM") as pps,
                tc.tile_pool(name="pp", bufs=3) as pp,
                tc.tile_pool(name="ep", bufs=2) as ep,
            ):
                def qproj(s):
                    rq = srng.tile([128, 1536], f32, tag="r", name=f"rq{s}")
                    for c in range(NCT):
                        nc.tensor.matmul(
                            rq[:, 0:512],
                            w3[:, 0, c, :],
                            xt[:, c, s * 512 : (s + 1) * 512],
                            start=(c == 0),
                            stop=(c == NCT - 1),
                        )
                    nc.vector.tensor_copy(qts[s][:], rq[:, 0:512])

                # group list per (slot, phase): tiles split into runs of <=3
                def groups_of(ntiles):
                    out, i = [], 0
                    while i < ntiles:
                        n = min(3, ntiles - i)
                        out.append((i, n))
                        i += n
                    return out

                # a group: score matmuls -> batched exp -> (mask) -> p_sb
                def emit_scores(s, g0, n, own, mask_lo):
                    kt = ktq if own else ktpar
                    S = srng.tile([128, 1536], f32, tag="r", name="sg")
                    for i in range(n):
                        t = g0 + i  # tile index within phase (128 cols each)
                        nc.tensor.matmul(
                            S[:, i * 512 : (i + 1) * 512],
                            kt[:, t * 128 : (t + 1) * 128],
                            qts[s][:],
                            start=True,
                            stop=True,
                        )
                    p = pp.tile([128, 1536], bf, tag="p", name="pg")
                    nc.scalar.activation(p[:, 0 : n * 512], S[:, 0 : n * 512], EXP)
                    if mask_lo is not None:
                        mtile = mo if own else mp
                        for i in range(n):
                            t = g0 + i
                            if t >= mask_lo:
                                j = t - mask_lo
                                nc.vector.tensor_mul(
                                    p[:, i * 512 : (i + 1) * 512],
                                    p[:, i * 512 : (i + 1) * 512],
                                    mtile[:, j * 512 : (j + 1) * 512],
                                )
                    return p

                # AV + column-packed denominator partials for a group
                def emit_av(s, g0, n, own, p, first):
                    v3 = v3o if own else v3par
                    P = pps.tile([128, 1024], f32, tag="pv", name="pv")
                    for i in range(n):
                        t = g0 + i
                        nc.tensor.matmul(
                            P[:, 0:512],
                            v3[:, t, :],
                            p[:, i * 512 : (i + 1) * 512],
                            start=(i == 0),
                            stop=(i == n - 1),
                            skip_group_check=True,
                        )
                    for i in range(4):
                        mv = (
                            p[:, i * 512 : (i + 1) * 512]
                            if i < n
                            else wut[:, 0:512]
                        )
                        nc.tensor.matmul(
                            P[32 * i : 32 * i + 32, 512:1024],
                            on32[:],
                            mv,
                            start=True,
                            stop=True,
                            skip_group_check=True,
                            tile_position=(0, 32 * i),
                        )
                    if first:
                        nc.vector.tensor_copy(accs[s][:], P[:, 0:1024])
                    else:
                        nc.vector.tensor_add(accs[s][:], accs[s][:], P[:, 0:1024])

                # software pipeline: emit scores(g+1) before av(g)
                def emit_phase(s, own, pending):
                    ntiles = 4 * (s + 1)
                    mask_lo = ntiles - 4
                    first = own
                    for g0, n in groups_of(ntiles):
                        p = emit_scores(s, g0, n, own, mask_lo)
                        if pending is not None:
                            emit_av(*pending)
                        pending = (s, g0, n, own, p, first and g0 == 0)
                    return pending

                pending = None
                for s in range(4):
                    qproj(s)
                    pending = emit_phase(s, True, pending)
                emit_blends()
                for s in (3, 2, 1, 0):
                    pending = emit_phase(s, False, pending)
                    if pending is not None:  # flush so acc[s] is complete
                        emit_av(*pending)
                        pending = None
                    # epilogue: broadcast-denominator matmul, reciprocal, scale
                    FD = srng.tile([128, 1536], f32, tag="r", name=f"fd{s}")
                    nc.tensor.matmul(
                        FD[:, 0:512],
                        scb[:],
                        accs[s][:, 512:1024],
                        start=True,
                        stop=True,
                    )
                    fdc = ep.tile([128, 512], f32, tag="fdc", name=f"fdc{s}")
                    nc.vector.tensor_copy(fdc[:], FD[:, 0:512])
                    rb = ep.tile([128, 512], f32, tag="rb", name=f"rb{s}")
                    nc.vector.reciprocal_approx_fast(rb[:], fdc[:])
                    ot = ep.tile([128, 512], f32, tag="ot", name=f"ot{s}")
                    nc.vector.tensor_mul(ot[:], accs[s][:, 0:512], rb[:])
                    nc.gpsimd.dma_start(out_d[s], ot[:])

    nc.compile()
    return nc


def _role_qtiles(h):
    return QTILES_A if h == 0 else QTILES_B


def _diag_mask():
    """[128, 4*512] bf16: tile j of the diagonal 512-chunk, k<=q."""
    m = np.zeros((128, 4, 512), np.float32)
    k = np.arange(128)[:, None]
    q = np.arange(512)[None, :]
    for j in range(4):
        m[:, j, :] = (128 * j + k <= q).astype(np.float32)
    return np.ascontiguousarray(m.reshape(128, 2048)).astype(BF16)


def make_in_maps(x, Wq, Wk, Wv):
    """Host-side sharding + layout prep. x [B,T,C] f32, W* [C,HS] f32."""
    wq_s = np.asarray(Wq, np.float32) / math.sqrt(HS)
    w3 = np.stack(
        [wq_s, np.asarray(Wk, np.float32), np.asarray(Wv, np.float32)]
    )
    w3_arr = np.ascontiguousarray(
        w3.reshape(3, NCT, 128, HS).transpose(2, 0, 1, 3)
    ).astype(BF16)
    mo = _diag_mask()
    mp_by_role = [
        np.zeros((128, 2048), BF16),  # role A: partner diag chunk fully masked
        np.ones((128, 2048), BF16),   # role B: partner chunks fully visible
    ]

    in_maps = []
    for core in range(NCORES):
        b, h = core // 2, core % 2
        qtiles = _role_qtiles(h)
        rows = np.concatenate(
            [np.arange(g * 512, (g + 1) * 512) for g in qtiles]
        )
        xr = np.asarray(x[b])[rows]  # [2048 rows, C] f32
        xT = np.ascontiguousarray(xr.T).astype(BF16)  # [C, 2048]
        xt_arr = np.ascontiguousarray(
            xT.reshape(NCT, 128, TOWN).transpose(1, 0, 2)
        )  # [128, NCT, 2048]
        in_maps.append(
            {
                "xt": xt_arr,
                "w3": w3_arr,
                "mo": mo,
                "mp": mp_by_role[h],
                # partner parity selectors: partner parity = 1-h
                "ps0": np.full((128, 1), float(h), np.float32),
                "ps1": np.full((128, 1), float(1 - h), np.float32),
            }
        )
    return in_maps


def assemble_out(results):
    """results: list of 8 dicts with 'out' [4,128,512] -> y [B,T,HS] f32."""
    y = np.zeros((B, T, HS), np.float32)
    for core in range(NCORES):
        b, h = core // 2, core % 2
        qtiles = _role_qtiles(h)
        o = np.asarray(results[core]["out"])  # [4, 128, 512] = y^T per slot
        for s in range(4):
            g = qtiles[s]
            y[b, g * 512 : (g + 1) * 512] = o[s].T
    return y


_NC_CACHE = None


def _get_graph():
    global _NC_CACHE
    if _NC_CACHE is None:
        _NC_CACHE = build_graph()
    return _NC_CACHE


def kernel(x, Wq, Wk, Wv):
    import time

    nc = _get_graph()
    in_maps = make_in_maps(x, Wq, Wk, Wv)
    try:
        res = run_bass_kernel_spmd(nc, in_maps, list(range(NCORES)))
    except Exception:
        time.sleep(15)  # transient device/mesh hiccup: one retry
        res = run_bass_kernel_spmd(nc, in_maps, list(range(NCORES)))
    return assemble_out(res.results)


# revision 13
# speedup vs baseline: 52.4191x; 1.0888x over previous
"""Trainium2 Bass kernel for single-head causal attention (B=4, T=4096, C=2048, HS=128).

Sharding: 2 cores per batch element (8 cores, B=4), interleaved 512-row
q-chunks: role A (even cores) owns global chunks [0,2,4,6], role B (odd)
[1,3,5,7]. Each core projects Q^T/K^T/V^T for its own 2048 rows, AllGathers
K^T/V^T within its pair via a DRAM staging buffer, and runs causal
attention over its 4 q-slots with static per-slot extents of 2(s+1)
512-chunks.

Attention is split own-chunks-first / partner-chunks-second so the
AllGather overlaps Q projections + the own half of attention. Scores for
3 k-tiles at a time land in a 3-bank PSUM tile and are exponentiated by a
single batched ACT instruction; AV partials and column-packed (tile_position)
denominator partials accumulate per group in a 2-bank PSUM tile and are
folded into per-slot SBUF accumulators by the vector engine. Softmax
normalization uses a broadcast matmul (ones/32 stationary) +
reciprocal_approx_fast. The output is written as y^T [slot, hs, 512] and
transposed on the host. Partner data addressing is role-independent: both
parities of the gathered buffer are loaded and blended with per-core 0/1
selectors, so all 8 cores run one SPMD graph.
"""

import math

import numpy as np
import ml_dtypes

import concourse.bacc as bacc
import concourse.tile as tile
from concourse import mybir
from concourse.bass_utils import run_bass_kernel_spmd

B, T, C, HS = 4, 4096, 2048, 128
NCORES = 8
TOWN = 2048              # sequence rows owned per core
NCT = C // 128           # 16 contraction tiles
QTILES_A = [0, 2, 4, 6]
QTILES_B = [1, 3, 5, 7]

BF16 = ml_dtypes.bfloat16


def build_graph(with_collective=True):
    nc = bacc.Bacc(
        "TRN2", target_bir_lowering=False, debug=False, num_devices=NCORES
    )
    bf = mybir.dt.bfloat16
    f32 = mybir.dt.float32
    EXP = mybir.ActivationFunctionType.Exp

    xt_d = nc.dram_tensor("xt", [128, NCT, TOWN], bf, kind="ExternalInput")
    w3_d = nc.dram_tensor("w3", [128, 3, NCT, HS], bf, kind="ExternalInput")
    mo_d = nc.dram_tensor("mo", [128, 2048], bf, kind="ExternalInput")
    mp_d = nc.dram_tensor("mp", [128, 2048], bf, kind="ExternalInput")
    ps0_d = nc.dram_tensor("ps0", [128, 1], f32, kind="ExternalInput")
    ps1_d = nc.dram_tensor("ps1", [128, 1], f32, kind="ExternalInput")
    # y^T per slot, normalized; host transposes to [512, HS]
    out_d = nc.dram_tensor("out", [4, 128, 512], f32, kind="ExternalOutput")

    with tile.TileContext(nc) as tc:
        with (
            tc.tile_pool(name="big", bufs=1) as big,
            tc.tile_pool(name="dram", bufs=1, space="DRAM") as dram,
        ):
            # ---- loads, spread across the three DMA paths ----
            w3 = big.tile([128, 3, NCT, HS], bf, tag="w3")
            xt = big.tile([128, NCT, TOWN], bf, tag="xt")

            def xchunk(eng, ch):
                eng.dma_start(
                    xt[:, ch * 2 : (ch + 1) * 2, :],
                    xt_d[:, ch * 2 : (ch + 1) * 2, :],
                )

            nc.scalar.dma_start(w3[:, 1:3, 0:4, :], w3_d[:, 1:3, 0:4, :])
            # x c-tiles round-robin across the three DMA queues in
            # consumption order (fine granularity avoids PE starvation);
            # the w3 K/V tail is split in two on scalar between its x tiles
            qs = [nc.sync, nc.scalar, nc.gpsimd]
            for c in range(NCT):
                qs[c % 3].dma_start(xt[:, c : c + 1, :], xt_d[:, c : c + 1, :])
                if c == 1:
                    nc.scalar.dma_start(
                        w3[:, 1:3, 4:10, :], w3_d[:, 1:3, 4:10, :]
                    )
                elif c == 4:
                    nc.scalar.dma_start(
                        w3[:, 1:3, 10:16, :], w3_d[:, 1:3, 10:16, :]
                    )
            nc.scalar.dma_start(w3[:, 0:1, :, :], w3_d[:, 0:1, :, :])
            ps0 = big.tile([128, 1], f32, tag="ps0")
            nc.gpsimd.dma_start(ps0[:], ps0_d[:])
            ps1 = big.tile([128, 1], f32, tag="ps1")
            nc.gpsimd.dma_start(ps1[:], ps1_d[:])
            mo = big.tile([128, 2048], bf, tag="mo")
            nc.gpsimd.dma_start(mo[:], mo_d[:])
            mp = big.tile([128, 2048], bf, tag="mp")
            nc.gpsimd.dma_start(mp[:], mp_d[:])

            # ---- constants ----
            wut = big.tile([128, 512], bf, tag="wut")
            nc.vector.memset(wut[:], 0.0)
            on32 = big.tile([128, 32], bf, tag="on32")
            nc.vector.memset(on32[:], 1.0)
            scb = big.tile([128, 128], f32, tag="scb")
            nc.vector.memset(scb[:], 1.0 / 32.0)

            # ---- persistent SBUF tensors ----
            ktq = big.tile([128, TOWN], bf, tag="ktq")  # own K^T (slot order)
            vtq = big.tile([128, TOWN], bf, tag="vtq")  # own V^T
            v3o = big.tile([128, 16, HS], bf, tag="v3o")  # own V (k-major)
            qts = [
                big.tile([128, 512], bf, tag=f"qt{s}", name=f"qt{s}")
                for s in range(4)
            ]
            ktp = [
                big.tile([128, TOWN], bf, tag=f"ktp{r}", name=f"ktp{r}")
                for r in range(2)
            ]
            v3p = [
                big.tile([128, 16, HS], bf, tag=f"v3p{r}", name=f"v3p{r}")
                for r in range(2)
            ]
            ktpar = big.tile([128, TOWN], bf, tag="ktpar")
            v3par = big.tile([128, 16, HS], bf, tag="v3par")
            accs = [
                big.tile([128, 1024], f32, tag=f"acc{s}", name=f"acc{s}")
                for s in range(4)
            ]

            kvb = dram.tile([256, TOWN], bf, tag="kvb")
            kvg = dram.tile([512, TOWN], bf, tag="kvg")
            # separate staging for the own-V transpose: reads of the
            # collective's input buffer get serialized behind the collective
            vst = dram.tile([128, TOWN], bf, tag="vst")

            # ---- projections: K,V for own rows ----
            with tc.tile_pool(name="pjps", bufs=8, space="PSUM") as pjps:
                # PE warm-up while the first input DMAs land
                wup = pjps.tile([128, 512], f32, tag="pj", name="wup")
                for _ in range(28):
                    nc.tensor.matmul(
                        wup[:], wut[:, 0:128], wut[:], start=True, stop=True
                    )
                ps8 = [
                    pjps.tile([128, 512], f32, tag="pj", name=f"pa{i}")
                    for i in range(8)
                ]
                for c in range(NCT):
                    for wi in range(2):  # 0 = K, 1 = V
                        for t4 in range(4):
                            nc.tensor.matmul(
                                ps8[wi * 4 + t4][:],
                                w3[:, wi + 1, c, :],
                                xt[:, c, t4 * 512 : (t4 + 1) * 512],
                                start=(c == 0),
                                stop=(c == NCT - 1),
                            )
                for wi, dest in [(0, ktq), (1, vtq)]:
                    for t4 in range(4):
                        nc.vector.tensor_copy(
                            dest[:, t4 * 512 : (t4 + 1) * 512],
                            ps8[wi * 4 + t4][:],
                        )
                nc.sync.dma_start(kvb[0:128, :], ktq[:])
                nc.sync.dma_start(kvb[128:256, :], vtq[:])
                # own V transpose via a private staging buffer, emitted
                # before the collective so it is not serialized behind it
                nc.sync.dma_start(vst[:], vtq[:])
                nc.sync.dma_start_transpose(v3o[:], vst[:])
                if with_collective:
                    nc.gpsimd.collective_compute(
                        "AllGather",
                        mybir.AluOpType.bypass,
                        replica_groups=[[0, 1], [2, 3], [4, 5], [6, 7]],
                        ins=[kvb.opt()],
                        outs=[kvg.opt()],
                    )
                else:  # timeline-model stub: same data volume, no comms
                    nc.scalar.dma_start(kvg[0:256, :], kvb[:])
                    nc.scalar.dma_start(kvg[256:512, :], kvb[:])
                # partner halves (block on the collective; dedicated queues)
                nc.gpsimd.dma_start(ktp[0][:], kvg[0:128, :])
                nc.gpsimd.dma_start(ktp[1][:], kvg[256:384, :])
                nc.sync.dma_start_transpose(v3p[0][:], kvg[128:256, :])
                nc.sync.dma_start_transpose(v3p[1][:], kvg[384:512, :])

            # role-independent partner buffers: blend the two parities with
            # per-core 0/1 selectors (ps0 = partner-is-parity-0). Emitted
            # between the own and partner phases so the vector queue is not
            # head-of-line blocked on the collective during the own phase.
            def emit_blends():
                nc.vector.tensor_scalar_mul(ktp[0][:], ktp[0][:], ps0[:])
                nc.vector.tensor_scalar_mul(ktp[1][:], ktp[1][:], ps1[:])
                nc.vector.tensor_add(ktpar[:], ktp[0][:], ktp[1][:])
                nc.vector.tensor_scalar_mul(v3p[0][:], v3p[0][:], ps0[:])
                nc.vector.tensor_scalar_mul(v3p[1][:], v3p[1][:], ps1[:])
                nc.vector.tensor_add(v3par[:], v3p[0][:], v3p[1][:])

            # ---- attention ----
            with (
                tc.tile_pool(name="srng", bufs=2, space="PSUM") as srng,
                tc.tile_pool(name="pps", bufs=1, space="PSUM") as pps,
                tc.tile_pool(name="pp", bufs=3) as pp,
                tc.tile_pool(name="ep", bufs=2) as ep,
            ):
                def qproj(s):
                    rq = srng.tile([128, 1536], f32, tag="r", name=f"rq{s}")
                    for c in range(NCT):
                        nc.tensor.matmul(
                            rq[:, 0:512],
                            w3[:, 0, c, :],
                            xt[:, c, s * 512 : (s + 1) * 512],
                            start=(c == 0),
                            stop=(c == NCT - 1),
                        )
                    nc.vector.tensor_copy(qts[s][:], rq[:, 0:512])

                # group list per (slot, phase): tiles split into runs of <=3
                def groups_of(ntiles):
                    out, i = [], 0
                    while i < ntiles:
                        n = min(3, ntiles - i)
                        out.append((i, n))
                        i += n
                    return out

                # a group: score matmuls -> batched exp -> (mask) -> p_sb
                def emit_scores(s, g0, n, own, mask_lo):
                    kt = ktq if own else ktpar
                    S = srng.tile([128, 1536], f32, tag="r", name="sg")
                    for i in range(n):
                        t = g0 + i  # tile index within phase (128 cols each)
                        nc.tensor.matmul(
                            S[:, i * 512 : (i + 1) * 512],
                            kt[:, t * 128 : (t + 1) * 128],
                            qts[s][:],
                            start=True,
                            stop=True,
                        )
                    p = pp.tile([128, 1536], bf, tag="p", name="pg")
                    nc.scalar.activation(p[:, 0 : n * 512], S[:, 0 : n * 512], EXP)
                    if mask_lo is not None:
                        mtile = mo if own else mp
                        for i in range(n):
                            t = g0 + i
                            if t >= mask_lo:
                                j = t - mask_lo
                                nc.vector.tensor_mul(
                                    p[:, i * 512 : (i + 1) * 512],
                                    p[:, i * 512 : (i + 1) * 512],
                                    mtile[:, j * 512 : (j + 1) * 512],
                                )
                    return p

                # AV + column-packed denominator partials for a group
                def emit_av(s, g0, n, own, p, first):
                    v3 = v3o if own else v3par
                    P = pps.tile([128, 1024], f32, tag="pv", name="pv")
                    for i in range(n):
                        t = g0 + i
                        nc.tensor.matmul(
                            P[:, 0:512],
                            v3[:, t, :],
                            p[:, i * 512 : (i + 1) * 512],
                            start=(i == 0),
                            stop=(i == n - 1),
                            skip_group_check=True,
                        )
                    for i in range(4):
                        mv = (
                            p[:, i * 512 : (i + 1) * 512]
                            if i < n
                            else wut[:, 0:512]
                        )
                        nc.tensor.matmul(
                            P[32 * i : 32 * i + 32, 512:1024],
                            on32[:],
                            mv,
                            start=True,
                            stop=True,
                            skip_group_check=True,
                            tile_position=(0, 32 * i),
                        )
                    if first:
                        nc.vector.tensor_copy(accs[s][:], P[:, 0:1024])
                    else:
                        nc.vector.tensor_add(accs[s][:], accs[s][:], P[:, 0:1024])

                # software pipeline: emit scores(g+1) before av(g)
                def emit_phase(s, own, pending):
                    ntiles = 4 * (s + 1)
                    mask_lo = ntiles - 4
                    first = own
                    for g0, n in groups_of(ntiles):
                        p = emit_scores(s, g0, n, own, mask_lo)
                        if pending is not None:
                            emit_av(*pending)
                        pending = (s, g0, n, own, p, first and g0 == 0)
                    return pending

                pending = None
                for s in range(4):
                    qproj(s)
                    pending = emit_phase(s, True, pending)
                emit_blends()
                for s in (3, 2, 1, 0):
                    pending = emit_phase(s, False, pending)
                    if pending is not None:  # flush so acc[s] is complete
                        emit_av(*pending)
                        pending = None
                    # epilogue: broadcast-denominator matmul, reciprocal, scale
                    FD = srng.tile([128, 1536], f32, tag="r", name=f"fd{s}")
                    nc.tensor.matmul(
                        FD[:, 0:512],
                        scb[:],
                        accs[s][:, 512:1024],
                        start=True,
                        stop=True,
                    )
                    fdc = ep.tile([128, 512], f32, tag="fdc", name=f"fdc{s}")
                    nc.vector.tensor_copy(fdc[:], FD[:, 0:512])
                    rb = ep.tile([128, 512], f32, tag="rb", name=f"rb{s}")
                    nc.vector.reciprocal_approx_fast(rb[:], fdc[:])
                    ot = ep.tile([128, 512], f32, tag="ot", name=f"ot{s}")
                    nc.vector.tensor_mul(ot[:], accs[s][:, 0:512], rb[:])
                    nc.gpsimd.dma_start(out_d[s], ot[:])

    nc.compile()
    return nc


def _role_qtiles(h):
    return QTILES_A if h == 0 else QTILES_B


def _diag_mask():
    """[128, 4*512] bf16: tile j of the diagonal 512-chunk, k<=q."""
    m = np.zeros((128, 4, 512), np.float32)
    k = np.arange(128)[:, None]
    q = np.arange(512)[None, :]
    for j in range(4):
        m[:, j, :] = (128 * j + k <= q).astype(np.float32)
    return np.ascontiguousarray(m.reshape(128, 2048)).astype(BF16)


def make_in_maps(x, Wq, Wk, Wv):
    """Host-side sharding + layout prep. x [B,T,C] f32, W* [C,HS] f32."""
    wq_s = np.asarray(Wq, np.float32) / math.sqrt(HS)
    w3 = np.stack(
        [wq_s, np.asarray(Wk, np.float32), np.asarray(Wv, np.float32)]
    )
    w3_arr = np.ascontiguousarray(
        w3.reshape(3, NCT, 128, HS).transpose(2, 0, 1, 3)
    ).astype(BF16)
    mo = _diag_mask()
    mp_by_role = [
        np.zeros((128, 2048), BF16),  # role A: partner diag chunk fully masked
        np.ones((128, 2048), BF16),   # role B: partner chunks fully visible
    ]

    in_maps = []
    for core in range(NCORES):
        b, h = core // 2, core % 2
        qtiles = _role_qtiles(h)
        rows = np.concatenate(
            [np.arange(g * 512, (g + 1) * 512) for g in qtiles]
        )
        xr = np.asarray(x[b])[rows]  # [2048 rows, C] f32
        xT = np.ascontiguousarray(xr.T).astype(BF16)  # [C, 2048]
        xt_arr = np.ascontiguousarray(
            xT.reshape(NCT, 128, TOWN).transpose(1, 0, 2)
        )  # [128, NCT, 2048]
        in_maps.append(
            {
                "xt": xt_arr,
                "w3": w3_arr,
                "mo": mo,
                "mp": mp_by_role[h],
                # partner parity selectors: partner parity = 1-h
                "ps0": np.full((128, 1), float(h), np.float32),
                "ps1": np.full((128, 1), float(1 - h), np.float32),
            }
        )
    return in_maps


def assemble_out(results):
    """results: list of 8 dicts with 'out' [4,128,512] -> y [B,T,HS] f32."""
    y = np.zeros((B, T, HS), np.float32)
    for core in range(NCORES):
        b, h = core // 2, core % 2
        qtiles = _role_qtiles(h)
        o = np.asarray(results[core]["out"])  # [4, 128, 512] = y^T per slot
        for s in range(4):
            g = qtiles[s]
            y[b, g * 512 : (g + 1) * 512] = o[s].T
    return y


_NC_CACHE = None


def _get_graph():
    global _NC_CACHE
    if _NC_CACHE is None:
        _NC_CACHE = build_graph()
    return _NC_CACHE


def kernel(x, Wq, Wk, Wv):
    import time

    nc = _get_graph()
    in_maps = make_in_maps(x, Wq, Wk, Wv)
    try:
        res = run_bass_kernel_spmd(nc, in_maps, list(range(NCORES)))
    except Exception:
        time.sleep(15)  # transient device/mesh hiccup: one retry
        res = run_bass_kernel_spmd(nc, in_maps, list(range(NCORES)))
    return assemble_out(res.results)
